# revision 13
# baseline (speedup 1.0000x reference)
"""Trainium2 Bass kernel for BaselineDNN pooling problem.

Per core (512 of 4096 batch rows, data-parallel across 8 cores):
  1. dma_gather (InstDMAGatherAnt ucode, NI=1024 HW limit per instruction)
     fetches embedding rows from a per-group host-compacted table
     ([~23k unique rows, 768B stride]) into [128 rows, 24 tokens, EP]
     SBUF tiles, THREE gathers per tile (slot ranges 0:8/8:16/16:24),
     plus one 8-slot tail tile; 25 gathers per group rotating over 4
     SWDGE queues so desc-gen overlaps DMA drain.
  2. DVE TT-add tree: per tile-pair, fold 24 slots -> 12 (3 ops at 3600
     elems, bf16 2x mode); binary counter above; token-fold 12->6->3->1
     with f32 tail -> mean pool (x 1/len on ACT).
  3. Same TT-max tree over valid tiles only (rows length-sorted on host
     so each 128-row group has a tight valid band; boundary masked at
     8-token granularity by adding -1e30 via broadcast tensor_tensor
     before the max ops).
  4. PE transposes rep ([128,600] -> [600,128] in 120-col chunks) into
     per-group rep_T tiles ([120, 128], 5 k-chunks).
  5. MLP on PE (h_T = relu(W1_T @ rep_T + b1), out_T = W2_T @ h_T + b2)
  6. out_T [3,512] DMA'd out; host inverts the row permutation.

Self-contained: hardcodes all shapes from the problem spec.
"""

import numpy as np
from contextlib import ExitStack

import ml_dtypes

import concourse.bacc as bacc
import concourse.tile as tile
from concourse import mybir
from concourse.bass_utils import run_bass_kernel_spmd
from concourse.masks import make_identity

VOCAB, DIM = 100000, 300
B, L = 4096, 200
HIDDEN, OUT = 1000, 3
NCORES = 8
P = 128
RPC = B // NCORES            # 512 rows per core
G = RPC // P                 # 4 groups of 128 rows
TCH = 8                      # tokens per gather instruction (NI=1024 limit)
NI = P * TCH                 # 1024 indices per gather
CW = NI // 16                # 64 idx columns per gather chunk
NCH = L // TCH               # 25 gather chunks per group
TS = 24                      # token slots per big tile (3 gathers)
NBT = (L - TCH) // TS        # 8 big tiles per group
TAIL = L - TS * NBT          # 8 tail slots (1 gather)
HT = TS // 2                 # 12 folded slots per pair node
NEG = -1.0e30
NQ = 4                       # SWDGE queues

EP = 384                     # bf16 row padded to 768B (256B-divisible)

K1 = 120                     # rep contraction chunk (600 = 5*120)
NK1 = (2 * DIM) // K1        # 5
MJ = 125                     # hidden m-chunk (1000 = 8*125)
NJ = HIDDEN // MJ            # 8

F32 = mybir.dt.float32
BF16 = mybir.dt.bfloat16
GDT = BF16
GNP = ml_dtypes.bfloat16
I16 = mybir.dt.int16
ALU = mybir.AluOpType
ACT_F = mybir.ActivationFunctionType

_BUILD_CACHE = {}


def _build(lhi, llo, vg):
    """Emit the SPMD program. lhi/llo: per-group max/min valid length;
    vg: padded per-group compact-table row count (identical across cores
    by construction)."""
    nc = bacc.Bacc(
        "TRN2", target_bir_lowering=False, debug=False, enable_asserts=False,
        num_swdge_queues=NQ,
    )
    gtab = nc.dram_tensor("gtab", [G, vg, EP], GDT, kind="ExternalInput")
    xg = nc.dram_tensor("xg", [G, P, NCH * CW], I16, kind="ExternalInput")
    aoff = nc.dram_tensor("aoff", [G, P, L], GDT, kind="ExternalInput")
    invlen = nc.dram_tensor("invlen", [G, P, 1], F32, kind="ExternalInput")
    w1 = nc.dram_tensor("w1", [2 * DIM, HIDDEN], BF16, kind="ExternalInput")
    b1 = nc.dram_tensor("b1", [HIDDEN], F32, kind="ExternalInput")
    w2 = nc.dram_tensor("w2", [HIDDEN, OUT], BF16, kind="ExternalInput")
    b2 = nc.dram_tensor("b2", [OUT], F32, kind="ExternalInput")
    out_t = nc.dram_tensor("out_t", [OUT, RPC], F32, kind="ExternalOutput")

    with tile.TileContext(nc) as tc, ExitStack() as ctx:
        persist = ctx.enter_context(tc.tile_pool(name="persist", bufs=1))
        gpool = ctx.enter_context(tc.tile_pool(name="gpool", bufs=4))
        wpool = ctx.enter_context(tc.tile_pool(name="wpool", bufs=3))
        apool = ctx.enter_context(tc.tile_pool(name="apool", bufs=2))
        mpool = ctx.enter_context(tc.tile_pool(name="mpool", bufs=2))
        rpool = ctx.enter_context(tc.tile_pool(name="rpool", bufs=2))
        ppool = ctx.enter_context(tc.tile_pool(name="ppool", bufs=2, space="PSUM"))
        hpool = ctx.enter_context(tc.tile_pool(name="hpool", bufs=2, space="PSUM"))
        opool = ctx.enter_context(tc.tile_pool(name="opool", bufs=2, space="PSUM"))

        ident = persist.tile([P, P], F32, tag="ident")
        make_identity(nc, ident[:])

        # group-0 small inputs first so its gathers start early
        xo_l = [mpool.tile([P, NCH * CW], I16, tag="xo", name=f"xo{g}")
                for g in range(G)]
        ao_l = [mpool.tile([P, L], GDT, tag="ao", name=f"ao{g}")
                for g in range(G)]
        il_l = [mpool.tile([P, 1], F32, tag="il", name=f"il{g}")
                for g in range(G)]
        nc.sync.dma_start(xo_l[0][:], xg[0])
        nc.sync.dma_start(ao_l[0][:], aoff[0])
        nc.sync.dma_start(il_l[0][:], invlen[0])

        # MLP weights/activations in bf16 (PE full rate)
        w1_t = [persist.tile([K1, HIDDEN], BF16, tag=f"w1_{k}", name=f"w1_{k}")
                for k in range(NK1)]
        for k in range(NK1):
            nc.sync.dma_start(w1_t[k][:], w1[k * K1:(k + 1) * K1, :])
        w2_t = [persist.tile([MJ, OUT], BF16, tag=f"w2_{j}", name=f"w2_{j}")
                for j in range(NJ)]
        b1_t = [persist.tile([MJ, 1], F32, tag=f"b1_{j}", name=f"b1_{j}")
                for j in range(NJ)]
        for j in range(NJ):
            nc.sync.dma_start(w2_t[j][:], w2[j * MJ:(j + 1) * MJ, :])
            nc.sync.dma_start(b1_t[j][:], b1[j * MJ:(j + 1) * MJ, None])
        b2_t = persist.tile([OUT, 1], F32, tag="b2")
        nc.sync.dma_start(b2_t[:], b2[:, None])

        ot_sb = persist.tile([OUT, RPC], F32, tag="ot", name="ot")

        HW_ = HT * EP                # flat elements per 12-slot half (4608)

        qn = 0
        for g in range(G):
            xo, ao, il = xo_l[g], ao_l[g], il_l[g]

            mv = min(-(-lhi[g] // TCH) * TCH, L)  # mask window end (8-tok gran)

            # ---- per-group flat accumulators ([P, 12 slots x EP], pads=0)
            acc_s = apool.tile([P, HW_], GDT, tag="accs", name="accs")
            acc_m = apool.tile([P, HW_], GDT, tag="accm", name="accm")

            def mask_tile(gv3, start, width):
                """Add -1e30 offsets to invalid (row, token) lanes in place."""
                clo = max(llo[g], start)
                chi = start + width
                if clo < chi:
                    n = chi - clo
                    sl = gv3[:, clo - start:chi - start, 0:DIM]
                    ab = ao[:, clo:chi].unsqueeze(2).broadcast_to([P, n, DIM])
                    nc.vector.tensor_tensor(out=sl, in0=sl, in1=ab, op=ALU.add)

            # ---- big tiles: 3 gathers each; flat 24->12 fold into accs.
            # All tree TTs read/write fully contiguous APs (incl. zero pads):
            # contiguous DVE ops run ~4x faster than EP-strided views.
            for i in range(NBT):
                gt = gpool.tile([P, TS * EP], GDT, tag="gt", name="gt")
                gv = gt[:].rearrange("p (t e) -> p t e", e=EP)
                for sub in range(TS // TCH):
                    c = i * (TS // TCH) + sub
                    nc.gpsimd.dma_gather(
                        gv[:, sub * TCH:(sub + 1) * TCH, :],
                        gtab[g],
                        xo[:, c * CW:(c + 1) * CW],
                        NI, NI, EP, queue_num=i % NQ,
                    )
                # queue == buffer slot: a slot's completion semaphore is
                # locked to one SWDGE queue, and slots rotate i % bufs
                if i == 0 and g + 1 < G:
                    # prefetch next group's small inputs
                    nc.sync.dma_start(xo_l[g + 1][:], xg[g + 1])
                    nc.sync.dma_start(ao_l[g + 1][:], aoff[g + 1])
                    nc.sync.dma_start(il_l[g + 1][:], invlen[g + 1])

                fl = gt[:]
                s0 = i * TS
                # sum fold (unmasked)
                if i == 0:
                    nc.vector.tensor_tensor(
                        out=acc_s[:], in0=fl[:, 0:HW_], in1=fl[:, HW_:2 * HW_],
                        op=ALU.add)
                else:
                    x = wpool.tile([P, HW_], GDT, tag="w", name="w")
                    nc.vector.tensor_tensor(
                        out=x[:], in0=fl[:, 0:HW_], in1=fl[:, HW_:2 * HW_],
                        op=ALU.add)
                    nc.vector.tensor_tensor(
                        out=acc_s[:], in0=acc_s[:], in1=x[:], op=ALU.add)
                # mask + max fold (participating tiles only)
                if s0 < mv:
                    mask_tile(gv, s0, TS)
                    if i == 0:
                        nc.vector.tensor_tensor(
                            out=acc_m[:], in0=fl[:, 0:HW_],
                            in1=fl[:, HW_:2 * HW_], op=ALU.max)
                    else:
                        x = wpool.tile([P, HW_], GDT, tag="w", name="w")
                        nc.vector.tensor_tensor(
                            out=x[:], in0=fl[:, 0:HW_], in1=fl[:, HW_:2 * HW_],
                            op=ALU.max)
                        nc.vector.tensor_tensor(
                            out=acc_m[:], in0=acc_m[:], in1=x[:], op=ALU.max)

            # ---- tail tile (tokens 192..199): sum always; max iff 192 < mv
            gtt = gpool.tile([P, TAIL * EP], GDT, tag="gtt", name="gtt", bufs=4)
            gvt = gtt[:].rearrange("p (t e) -> p t e", e=EP)
            nc.gpsimd.dma_gather(
                gvt, gtab[g], xo[:, NBT * (TS // TCH) * CW:], NI, NI, EP,
                queue_num=g % NQ,
            )
            flt = gtt[:]
            HWT = (TAIL // 2) * EP
            ta4 = wpool.tile([P, HWT], GDT, tag="w", name="w")
            nc.vector.tensor_tensor(
                out=ta4[:], in0=flt[:, 0:HWT], in1=flt[:, HWT:2 * HWT],
                op=ALU.add)
            ta2 = wpool.tile([P, 2 * EP], GDT, tag="tl", name="tl", bufs=2)
            nc.vector.tensor_tensor(
                out=ta2[:], in0=ta4[:][:, 0:2 * EP], in1=ta4[:][:, 2 * EP:],
                op=ALU.add)
            st = NBT * TS
            tam2 = None
            if st < mv:
                mask_tile(gvt, st, TAIL)
                tm4 = wpool.tile([P, HWT], GDT, tag="w", name="w")
                nc.vector.tensor_tensor(
                    out=tm4[:], in0=flt[:, 0:HWT], in1=flt[:, HWT:2 * HWT],
                    op=ALU.max)
                tam2 = wpool.tile([P, 2 * EP], GDT, tag="tl", name="tl",
                                  bufs=2)
                nc.vector.tensor_tensor(
                    out=tam2[:], in0=tm4[:][:, 0:2 * EP],
                    in1=tm4[:][:, 2 * EP:], op=ALU.max)

            def tfold(acc, op, out_f32, tail2):
                """Fold [P, 12*EP] acc (+[P, 2*EP] tail) to f32 [P, DIM]."""
                t6 = wpool.tile([P, 6 * EP], GDT, tag="w", name="w")
                nc.vector.tensor_tensor(
                    out=t6[:], in0=acc[:][:, 0:6 * EP], in1=acc[:][:, 6 * EP:],
                    op=op)
                t3 = wpool.tile([P, 3 * EP], GDT, tag="w", name="w")
                nc.vector.tensor_tensor(
                    out=t3[:], in0=t6[:][:, 0:3 * EP], in1=t6[:][:, 3 * EP:],
                    op=op)
                v = t3[:].rearrange("p (t e) -> p t e", e=EP)
                ov = out_f32.unsqueeze(1)
                nc.vector.tensor_tensor(
                    out=ov, in0=v[:, 0:1, 0:DIM], in1=v[:, 1:2, 0:DIM], op=op)
                nc.vector.tensor_tensor(
                    out=ov, in0=ov, in1=v[:, 2:3, 0:DIM], op=op)
                if tail2 is not None:
                    tv2 = tail2[:].rearrange("p (t e) -> p t e", e=EP)
                    nc.vector.tensor_tensor(
                        out=ov, in0=ov, in1=tv2[:, 0:1, 0:DIM], op=op)
                    nc.vector.tensor_tensor(
                        out=ov, in0=ov, in1=tv2[:, 1:2, 0:DIM], op=op)

            msum = mpool.tile([P, DIM], F32, tag="msum", name="msum")
            tfold(acc_s, ALU.add, msum[:], ta2)
            mean_t = mpool.tile([P, DIM], F32, tag="mean_t", name="mean_t")
            nc.scalar.mul(mean_t[:], msum[:], il[:, 0:1])

            mmax = mpool.tile([P, DIM], F32, tag="mmax", name="mmax")
            tfold(acc_m, ALU.max, mmax[:], tam2)

            # transpose mean (rep dims 0..299) and max (300..599) into
            # per-group rep_T tiles ([K1=120, P], 5 k-chunks). Chunk 2
            # straddles mean/max: assemble [128, 120] source via ACT
            # free-dim copies first (partition-offset writes are illegal).
            rep_g = [rpool.tile([K1, P], BF16, tag=f"rep{k}", name=f"rep{k}")
                     for k in range(NK1)]
            cat = rpool.tile([P, K1], F32, tag="cat", name="cat")
            nc.scalar.copy(out=cat[:, 0:60], in_=mean_t[:, 240:300])
            nc.scalar.copy(out=cat[:, 60:120], in_=mmax[:, 0:60])
            srcs = [mean_t[:, 0:120], mean_t[:, 120:240], cat[:],
                    mmax[:, 60:180], mmax[:, 180:300]]
            for k in range(NK1):
                pt = ppool.tile([K1, P], F32, tag="pt", name="pt")
                nc.tensor.transpose(out=pt[:], in_=srcs[k], identity=ident[:])
                nc.scalar.copy(out=rep_g[k][:], in_=pt[:])

            # per-group MLP on this group's 128 columns (overlaps later groups)
            gsl = slice(g * P, (g + 1) * P)
            h_g = [rpool.tile([MJ, P], BF16, tag=f"h{j}", name=f"h{j}")
                   for j in range(NJ)]
            for j in range(NJ):
                hp = hpool.tile([MJ, P], F32, tag="hp", name="hp")
                for k in range(NK1):
                    nc.tensor.matmul(
                        out=hp[:], lhsT=w1_t[k][:, j * MJ:(j + 1) * MJ],
                        rhs=rep_g[k][:], start=(k == 0), stop=(k == NK1 - 1),
                    )
                nc.scalar.activation(
                    out=h_g[j][:], in_=hp[:], func=ACT_F.Relu,
                    bias=b1_t[j][:, 0:1], scale=1.0,
                )
            op_ps = opool.tile([OUT, P], F32, tag="op", name="op")
            for j in range(NJ):
                nc.tensor.matmul(
                    out=op_ps[:], lhsT=w2_t[j][:], rhs=h_g[j][:],
                    start=(j == 0), stop=(j == NJ - 1),
                )
            nc.scalar.activation(
                out=ot_sb[:, gsl], in_=op_ps[:], func=ACT_F.Identity,
                bias=b2_t[:, 0:1], scale=1.0,
            )

        nc.sync.dma_start(out_t[:], ot_sb[:])

    nc.compile()

    # The tile scheduler reorders gathers, and SWDGE completion semaphores
    # are assigned round-robin over 8 DMASW lanes in FINAL stream order with
    # each lane locked to one queue. Reassign queue_num in stream order so
    # lane L always pairs with queue L % NQ.
    n = 0
    for f in nc.m.functions:
        for blk in f.blocks:
            for inst in blk.instructions:
                if isinstance(inst, mybir.InstDMAGatherAnt):
                    inst.queue_num = n % NQ
                    n += 1
    return nc


def _pack_idx16(idx_cg):
    """idx_cg: [P, L] group-local int indices. Returns [P, NCH*CW] int16
    (per chunk: NI-entry list in i = t*128 + p order, 16-partition
    wrapped idxs[i%16, i//16], replicated to 128 partitions)."""
    out = np.empty((P, NCH * CW), dtype=np.int16)
    for c in range(NCH):
        lst = idx_cg[:, c * TCH:(c + 1) * TCH].T.reshape(-1)  # [NI] t-major
        wrapped = lst.reshape(CW, 16).T                       # [16, CW]
        out[:, c * CW:(c + 1) * CW] = np.tile(wrapped, (P // 16, 1))
    return out


def _prepare(inputs):
    emb_np = np.asarray(inputs["emb_table"], dtype=np.float32)
    x_np = np.ascontiguousarray(np.asarray(inputs["x"])).astype(np.int64)
    lengths = np.asarray(inputs["lengths"]).astype(np.int64)
    w1_np = np.ascontiguousarray(np.asarray(inputs["W1"], dtype=np.float32).astype(ml_dtypes.bfloat16))
    b1_np = np.ascontiguousarray(np.asarray(inputs["b1"], dtype=np.float32))
    w2_np = np.ascontiguousarray(np.asarray(inputs["W2"], dtype=np.float32).astype(ml_dtypes.bfloat16))
    b2_np = np.ascontiguousarray(np.asarray(inputs["b2"], dtype=np.float32))

    # sort rows by length; rank r -> core r%8, slot r//8 so every core's
    # group g spans the same global length band (one SPMD program)
    order = np.argsort(lengths, kind="stable")
    rows_by_core = order.reshape(RPC, NCORES).T  # [8, 512]
    lens_cs = lengths[rows_by_core]              # [8, 512]
    lhi = tuple(int(lens_cs[:, g * P:(g + 1) * P].max()) for g in range(G))
    llo = tuple(int(lens_cs[:, g * P:(g + 1) * P].min()) for g in range(G))

    # per (core, group): compact table (unique rows) + int16 remapped idx
    uniqs, idx16s = [], []
    vg_req = 0
    for c in range(NCORES):
        rows = rows_by_core[c]
        for g in range(G):
            xg_blk = x_np[rows[g * P:(g + 1) * P]]          # [128, 200]
            uniq, inv = np.unique(xg_blk, return_inverse=True)
            assert len(uniq) < 32768, f"group table too large: {len(uniq)}"
            uniqs.append(uniq)
            idx16s.append(inv.reshape(P, L))
            vg_req = max(vg_req, len(uniq))
    vg = -(-vg_req // 16) * 16  # pad a little for alignment

    t_ar = np.arange(L)
    in_maps = []
    for c in range(NCORES):
        rows = rows_by_core[c]
        lc = lengths[rows]
        gtab = np.zeros((G, vg, EP), dtype=GNP)
        xg16 = np.empty((G, P, NCH * CW), dtype=np.int16)
        for g in range(G):
            uniq = uniqs[c * G + g]
            gtab[g, :len(uniq), :DIM] = emb_np[uniq].astype(GNP)
            xg16[g] = _pack_idx16(idx16s[c * G + g])
        ac = np.where(t_ar[None, :] < lc[:, None], GNP(0.0),
                      GNP(NEG)).astype(GNP).reshape(G, P, L)
        il = (1.0 / lc.astype(np.float64)).astype(np.float32).reshape(G, P, 1)
        in_maps.append({
            "gtab": gtab, "xg": xg16,
            "aoff": np.ascontiguousarray(ac), "invlen": np.ascontiguousarray(il),
            "w1": w1_np, "b1": b1_np, "w2": w2_np, "b2": b2_np,
        })
    return in_maps, rows_by_core, lhi, llo, vg


def run_with_results(inputs, trace=False, **kwargs):
    in_maps, rows_by_core, lhi, llo, vg = _prepare(inputs)
    key = (lhi, llo, vg)
    if key not in _BUILD_CACHE:
        _BUILD_CACHE[key] = _build(lhi, llo, vg)
    nc = _BUILD_CACHE[key]
    res = run_bass_kernel_spmd(
        nc, in_maps, core_ids=list(range(NCORES)), trace=trace, **kwargs
    )
    out = np.empty((B, OUT), np.float32)
    for c in range(NCORES):
        out[rows_by_core[c]] = np.asarray(res.results[c]["out_t"]).T
    return out, res


def kernel(**inputs) -> np.ndarray:
    out, _ = run_with_results(inputs, trace=False)
    return out
